# revision 7
# baseline (speedup 1.0000x reference)
"""Multi-head self-attention (B=4, N=2048, C=1024, H=16) on 8 NeuronCores.

Sharding: core = b*2 + g  (b in 0..3 batches, g in 0..1 head-groups of 8 heads).
Each core computes, for its batch b and its 8 heads:
    qkv slice -> causal attention -> partial out-projection (its heads' rows
    of Wout). Host adds the two head-group partials per batch and the bias.

Kernel layout notes:
  - everything transposed: x fed as xT [C, N]; Q^T/K^T kept as [d, n] so
    scores are computed as S^T[j, i] = K^T.T @ Q^T with softmax over j
    (partitions) done via PE (ones column appended to V gives the rowsum).
  - no max-subtraction in softmax: scores*0.125 are ~N(0,1), exp is safe,
    matching the reference's normalized result to fp32 rounding.
  - causal masking: lower-triangle j-tiles only; diagonal tiles get an
    additive -1e9 triangle via an identity-matmul into PSUM before exp.
  - matmuls in float32r (fp32 truncated to ~FP22): full PE rate at free
    dim >= 256, rel. error ~1e-4.
"""

import os
import sys
import types
import numpy as np

sys.path.insert(0, "/opt/trn_rl_repo")

B, N, C, H = 4, 2048, 1024, 16
D, HL = 64, 8          # head dim, heads per core
P = 128
CHUNK = 512            # i-chunk (query) width
NCH = N // CHUNK       # 4
CSL = HL * D           # 512, per-core qkv slice width
NEG = -1e9

TRACE = [False]        # test.py flips this for profiled runs
_cache = {}


def _install_ntff_hook():
    """Shim antenv.axon_hooks so trace=True can reach the NTFF profiler."""
    try:
        import antenv
        if "antenv.axon_hooks" in sys.modules:
            return
        mod = types.ModuleType("antenv.axon_hooks")
        _hook = {"fn": None}
        mod.set_axon_ntff_profile_hook = lambda fn: _hook.__setitem__("fn", fn)
        mod.get_axon_ntff_profile_hook = lambda: _hook["fn"]
        sys.modules["antenv.axon_hooks"] = mod
        antenv.axon_hooks = mod
        from trn_agent_boot.trn_boot import _ntff_profile_via_ctypes
        mod.set_axon_ntff_profile_hook(
            _ntff_profile_via_ctypes("/opt/axon/libaxon_pjrt.so"))
    except Exception:
        pass


def _build_nc():
    import concourse.bacc as bacc
    import concourse.tile as tile
    from concourse import mybir
    from contextlib import ExitStack

    f32 = mybir.dt.float32
    f32r = mybir.dt.float32r
    bf16 = mybir.dt.bfloat16
    Exp = mybir.ActivationFunctionType.Exp

    nc = bacc.Bacc("TRN2", target_bir_lowering=False)
    xT = nc.dram_tensor("xT", [C, N], f32, kind="ExternalInput")
    wq = nc.dram_tensor("wq", [C, CSL], f32, kind="ExternalInput")
    wk = nc.dram_tensor("wk", [C, CSL], f32, kind="ExternalInput")
    wv = nc.dram_tensor("wv", [C, CSL], f32, kind="ExternalInput")
    wo = nc.dram_tensor("wo", [CSL, C], f32, kind="ExternalInput")
    tri = nc.dram_tensor("tri", [P, P], bf16, kind="ExternalInput")
    ones64 = nc.dram_tensor("ones64", [1, 64], f32, kind="ExternalInput")
    vcol = nc.dram_tensor("vcol", [16, 8], f32, kind="ExternalInput")
    ident = nc.dram_tensor("ident", [P, P], bf16, kind="ExternalInput")
    out = nc.dram_tensor("out", [N, C], f32, kind="ExternalOutput")

    xTr = xT.rearrange("(t p) n -> p t n", p=P)       # [128, 8, 2048]
    wor = wo.rearrange("(s p) n -> p s n", p=P)       # [128, 4, 1024]

    KT = C // P  # 8 contraction tiles for the projections

    with tile.TileContext(nc) as tc, ExitStack() as ctx:
        perm = ctx.enter_context(tc.tile_pool(name="perm", bufs=1))
        qpool = ctx.enter_context(tc.tile_pool(name="qpool", bufs=2))
        apool = ctx.enter_context(tc.tile_pool(name="apool", bufs=2))
        xpool = ctx.enter_context(tc.tile_pool(name="xpool", bufs=2))
        wqk_pool = ctx.enter_context(tc.tile_pool(name="wqk", bufs=8))
        wv_pool = ctx.enter_context(tc.tile_pool(name="wvp", bufs=3))
        pt_pool = ctx.enter_context(tc.tile_pool(name="ptp", bufs=3))
        rc_pool = ctx.enter_context(tc.tile_pool(name="rcp", bufs=4))
        o_pool = ctx.enter_context(tc.tile_pool(name="opool", bufs=3))
        ps = ctx.enter_context(tc.tile_pool(name="ps", bufs=2, space="PSUM"))
        ps_pv = ctx.enter_context(tc.tile_pool(name="pspv", bufs=2, space="PSUM"))
        ps_o = ctx.enter_context(tc.tile_pool(name="pso", bufs=2, space="PSUM"))

        # persistent SBUF
        kT_sb = perm.tile([P, 4, N], f32r)            # K^T  (d x n), pair layout
        v_sb = perm.tile([P, N // P, HL, D + 1], f32r)  # V + ones column
        wo_sb = perm.tile([P, 4, C], f32r)
        tri_sb = perm.tile([P, P], bf16)
        id_sb = perm.tile([P, P], bf16)
        ones_sb = perm.tile([1, 64], f32r)

        nc.sync.dma_start(out=wo_sb, in_=wor.bitcast(f32r))
        nc.sync.dma_start(out=tri_sb, in_=tri[:, :])
        nc.sync.dma_start(out=id_sb, in_=ident[:, :])
        nc.sync.dma_start(out=ones_sb, in_=ones64[:, :].bitcast(f32r))
        vc = vcol[:, :]
        import concourse.bass as bass
        nc.sync.dma_start(
            out=v_sb[:, :, :, D],
            in_=bass.AP(tensor=vc.tensor, offset=vc.offset,
                        ap=[[0, P]] + [list(a) for a in vc.ap]).bitcast(f32r))

        for ic in range(NCH):
            cs = slice(ic * CHUNK, (ic + 1) * CHUNK)
            # ---- QKV projections for this chunk of tokens ----
            xt = xpool.tile([P, KT, CHUNK], f32r, tag="xt")
            nc.sync.dma_start(out=xt, in_=xTr[:, :, cs].bitcast(f32r))

            qt = qpool.tile([P, 4, CHUNK], f32r, tag="qt")
            for which, wmat, dest in (("q", wq, qt), ("k", wk, kT_sb)):
                for m in range(4):
                    sg = ps.tile([P, 2 * CHUNK], f32, tag="sg")
                    pq = sg[:, :CHUNK]
                    for ct in range(KT):
                        wt = wqk_pool.tile([P, P], f32r, tag="wqk")
                        nc.sync.dma_start(
                            out=wt,
                            in_=wmat[ct * P:(ct + 1) * P,
                                     m * P:(m + 1) * P].bitcast(f32r))
                        nc.tensor.matmul(pq, wt, xt[:, ct, :],
                                         start=(ct == 0), stop=(ct == KT - 1))
                    if which == "q":
                        nc.vector.tensor_copy(dest[:, m, :], pq)
                    else:
                        nc.vector.tensor_copy(dest[:, m, cs], pq)

            # V: two PSUM [128,1024] tiles hold the 4 row-tiles of this chunk
            vg = [ps.tile([P, 2 * CHUNK], f32, tag="sg", name=f"vg{i}") for i in range(2)]
            for ct in range(KT):
                wvt = wv_pool.tile([P, CHUNK], f32r, tag="wv")
                nc.sync.dma_start(
                    out=wvt,
                    in_=wv[ct * P:(ct + 1) * P, :].bitcast(f32r))
                for nt in range(4):
                    nc.tensor.matmul(
                        vg[nt // 2][:, (nt % 2) * CHUNK:(nt % 2 + 1) * CHUNK],
                        xt[:, ct, nt * P:(nt + 1) * P], wvt,
                        start=(ct == 0), stop=(ct == KT - 1))
            for nt in range(4):
                nc.vector.tensor_copy(
                    v_sb[:, ic * 4 + nt, :, 0:D],
                    vg[nt // 2][:, (nt % 2) * CHUNK:(nt % 2 + 1) * CHUNK]
                    .rearrange("p (h d) -> p h d", h=HL))

            # ---- causal attention for this chunk ----
            J = 4 * (ic + 1)            # j-tiles (keys) this chunk sees
            attnT = apool.tile([P, 4, CHUNK], f32r, tag="attnT")
            for hp in range(4):         # head pairs (2hp, 2hp+1)
                pv = [ps_pv.tile([D + 1, CHUNK], f32, tag="pv", name=f"pv{i}")
                      for i in range(2)]
                for gidx in range(J // 2):
                    sg = [ps.tile([P, 2 * CHUNK], f32, tag="sg", name=f"sg{i}")
                          for i in range(2)]
                    for slot in range(2):
                        jt = gidx * 2 + slot
                        s = jt - 4 * ic          # >=0: diagonal tile
                        off = 128 * s if s > 0 else 0
                        for hb in range(2):
                            pr = slice(hb * 64, hb * 64 + 64)
                            nc.tensor.matmul(
                                sg[hb][:, slot * CHUNK + off:(slot + 1) * CHUNK],
                                kT_sb[pr, hp, jt * P:(jt + 1) * P],
                                qt[pr, hp, off:CHUNK],
                                start=True, stop=(s < 0))
                            if s >= 0:
                                nc.tensor.matmul(
                                    sg[hb][:, slot * CHUNK + 128 * s:
                                           slot * CHUNK + 128 * s + P],
                                    id_sb, tri_sb, start=False, stop=True)
                    pt = [pt_pool.tile([P, 2 * CHUNK], f32r, tag="pt", name=f"pt{i}")
                          for i in range(2)]
                    for hb in range(2):
                        nc.scalar.activation(pt[hb][:], sg[hb][:], Exp,
                                             scale=0.125)
                    for slot in range(2):
                        jt = gidx * 2 + slot
                        s = jt - 4 * ic
                        off = 128 * s if s > 0 else 0
                        last = jt == J - 1
                        for hb in range(2):
                            nc.tensor.matmul(
                                pv[hb][:, off:CHUNK],
                                v_sb[:, jt, 2 * hp + hb, :],
                                pt[hb][:, slot * CHUNK + off:(slot + 1) * CHUNK],
                                start=(jt == 0), stop=last)
                # normalize: rows 0..63 are (P V)^T, row 64 is the rowsum
                for hb in range(2):
                    recip = rc_pool.tile([1, CHUNK], f32r, tag="rc")
                    with nc.allow_low_precision("fp32r recip: 13 mantissa bits is plenty here"):
                        nc.vector.reciprocal(recip, pv[hb][D:D + 1, :])
                    bc = ps_o.tile([64, CHUNK], f32, tag="bc")
                    nc.tensor.matmul(bc, ones_sb, recip, start=True, stop=True)
                    dst = attnT[hb * 64:hb * 64 + 64, hp, :]
                    nc.vector.tensor_copy(dst, pv[hb][0:D, :])
                    nc.vector.tensor_mul(dst, dst, bc)

            # ---- partial out-projection for this chunk ----
            for nt in range(4):
                for half in range(2):
                    po = ps_o.tile([P, CHUNK], f32, tag="bc")
                    for csub in range(4):
                        nc.tensor.matmul(
                            po, attnT[:, csub, nt * P:(nt + 1) * P],
                            wo_sb[:, csub, half * CHUNK:(half + 1) * CHUNK],
                            start=(csub == 0), stop=(csub == 3))
                    osb = o_pool.tile([P, CHUNK], f32, tag="o")
                    nc.vector.tensor_copy(osb, po)
                    nc.sync.dma_start(
                        out=out[ic * CHUNK + nt * P:ic * CHUNK + (nt + 1) * P,
                                half * CHUNK:(half + 1) * CHUNK],
                        in_=osb)

    nc.finalize()
    return nc


def kernel(x, attn_mask, Wqkv, Wout, bout):
    from concourse.bass_utils import run_bass_kernel_spmd
    import ml_dtypes

    if "nc" not in _cache:
        _install_ntff_hook()
        _cache["nc"] = _build_nc()
    nc = _cache["nc"]

    x = np.asarray(x, dtype=np.float32)
    Wqkv = np.asarray(Wqkv, dtype=np.float32)
    Wout = np.asarray(Wout, dtype=np.float32)
    bout = np.asarray(bout, dtype=np.float32)

    tri_np = np.where(np.arange(P)[:, None] > np.arange(P)[None, :],
                      np.float32(NEG), np.float32(0.0)).astype(ml_dtypes.bfloat16)
    id_np = np.eye(P, dtype=ml_dtypes.bfloat16)

    in_maps = []
    xTb = [np.ascontiguousarray(x[b].T) for b in range(B)]
    for core in range(8):
        b, g = divmod(core, 2)
        sl = slice(g * CSL, (g + 1) * CSL)
        in_maps.append({
            "xT": xTb[b],
            "wq": np.ascontiguousarray(Wqkv[:, :C][:, sl]),
            "wk": np.ascontiguousarray(Wqkv[:, C:2 * C][:, sl]),
            "wv": np.ascontiguousarray(Wqkv[:, 2 * C:][:, sl]),
            "wo": np.ascontiguousarray(Wout[sl, :]),
            "tri": tri_np,
            "ident": id_np,
            "ones64": np.ones((1, 64), dtype=np.float32),
            "vcol": np.ones((16, 8), dtype=np.float32),
        })

    res = run_bass_kernel_spmd(nc, in_maps, list(range(8)), trace=TRACE[0])
    _cache["last_result"] = res

    full = np.empty((B, N, C), dtype=np.float32)
    for b in range(B):
        full[b] = res.results[2 * b]["out"] + res.results[2 * b + 1]["out"] + bout
    return full
